# revision 26
# baseline (speedup 1.0000x reference)
"""Trainium2 kernel for nn_Graph_41609643163904.

The reference op is a sequential per-cell scatter sweep over a 48x48 grid.
Every step is linear, so the sweep is a fixed linear operator M (2304x2304)
of the weights.  M in x-major order is block-lower-triangular with NO decay
(dense 58%), so the v2/v3 dense blocked matmul costs 376 PE matmuls.

v4 exploits the exact SEMISEPARABLE structure of the cascade: all influence
of grid columns < c on columns >= c flows through a 96-dim state
s_c = [v_{c-1}; v_{c-2}], where v_c are the per-column "center read"
values.  Device work per core (1024 samples):
  * an 8-step chain computes anchor states (one per 2 output j-tiles);
    each step is ~3 matmuls (state transition + input injections),
  * each 128-row output j-tile = U_t @ s_anchor + a short banded window of
    direct input terms (~3 matmuls).
Total 200 matmuls @512 moving vs 752 for dense: ~2x less PE time.
All operands bf16 (measured end-to-end quantization error 4.3e-3 vs the
2e-2 budget); PSUM accumulates f32; output shipped bf16, upcast on host.

Data parallel over batch: 8192 samples / 8 cores, no cross-core comm.
"""

import os

import numpy as np
import ml_dtypes

SIZE = 48
D = 2
K = 5
N = SIZE * SIZE          # 2304
B = 8192
NCORES = 8
BS = B // NCORES         # 1024 samples per core

P = 128
NK = N // P              # 18 input k-tiles
NT = N // P              # 18 output j-tiles
MW = 512                 # moving width (fp32 PSUM bank limit)
NM = BS // MW            # 2 m-chunks per core

bf16 = ml_dtypes.bfloat16


# ---------------------------------------------------------------------------
# Host math: per-column operators and the state-space plan (fp64, exact).
# ---------------------------------------------------------------------------

def _col_ops(w):
    w = w.astype(np.float64)
    T, E, F1, F2, G1, G2 = {}, {}, {}, {}, {}, {}
    for c in range(2, SIZE - 2):
        L = np.zeros((SIZE, SIZE))
        for yp in range(2, SIZE - 2):
            for d in (1, 2):
                y = yp + d
                if 2 <= y <= SIZE - 3:
                    L[y, yp] = w[yp, c, 2 + d, 2]
        T[c] = np.linalg.inv(np.eye(SIZE) - L)

        Ec = np.zeros((SIZE, SIZE))
        for y in range(2, SIZE - 2):
            Ec[y, y] = w[y, c, 2, 2]
            for d in (1, 2):
                yp = y + d
                if yp <= SIZE - 3:
                    Ec[y, yp] = w[yp, c, 2 - d, 2]
        for y in (0, 1, SIZE - 2, SIZE - 1):
            Ec[y, y] = 1.0
            for yp in range(2, SIZE - 2):
                dy = y - yp
                if abs(dy) <= 2 and dy != 0:
                    Ec[y, yp] += w[yp, c, 2 + dy, 2]
        E[c] = Ec

        for mats, dx in ((F1, 3), (F2, 4), (G1, 1), (G2, 0)):
            A = np.zeros((SIZE, SIZE))
            for yp in range(2, SIZE - 2):
                for dy in range(-2, 3):
                    y = yp + dy
                    if 0 <= y < SIZE:
                        A[y, yp] = w[yp, c, 2 + dy, dx]
            mats[c] = A
    return T, E, F1, F2, G1, G2


def _propagate_v(T, F1, F2, c_from, c_to, n_rhs, vm1, vm2, inj):
    """Sensitivities of v_c (c in [c_from, c_to]) to a basis: v_{c_from-1}
    := vm1, v_{c_from-2} := vm2 (48 x n_rhs each, may be zero), plus
    per-column injections inj[c] (48 x n_rhs)."""
    Z = np.zeros((SIZE, n_rhs))
    v = {}

    def getv(c):
        if c in v:
            return v[c]
        if c == c_from - 1:
            return vm1
        if c == c_from - 2:
            return vm2
        return Z

    for c in range(c_from, c_to + 1):
        u = inj.get(c, Z)
        if c - 1 >= 2:
            u = u + F1[c - 1] @ getv(c - 1)
        if c - 2 >= 2:
            u = u + F2[c - 2] @ getv(c - 2)
        v[c] = T[c] @ u
    return v


def _build_plan(w):
    """Chain steps + band terms, all stationaries fp64 in lhsT layout
    (contract, out)."""
    T, E, F1, F2, G1, G2 = _col_ops(w)
    spans = [((P * t) // SIZE, (P * t + P - 1) // SIZE) for t in range(NT)]
    anc = [None, None] + [2 * ((t - 2) // 2) + 2 for t in range(2, NT)]

    I48 = np.eye(SIZE)
    Z48 = np.zeros((SIZE, SIZE))

    def out_rows(vm, inm, cols, n_rhs):
        Z = np.zeros((SIZE, n_rhs))
        outs = []
        for c in cols:
            if c < 2:
                o = inm.get(c, Z).copy()
                if c == 0:
                    o += G2[2] @ vm.get(2, Z)
                else:
                    o += G1[2] @ vm.get(2, Z) + G2[3] @ vm.get(3, Z)
            elif c > SIZE - 3:
                o = inm.get(c, Z).copy()
                if c == SIZE - 2:
                    o += F1[SIZE - 3] @ vm.get(SIZE - 3, Z)
                    o += F2[SIZE - 4] @ vm.get(SIZE - 4, Z)
                else:
                    o += F2[SIZE - 3] @ vm.get(SIZE - 3, Z)
            else:
                o = E[c] @ vm.get(c, Z)
                if c + 1 <= SIZE - 3:
                    o += G1[c + 1] @ vm.get(c + 1, Z)
                if c + 2 <= SIZE - 3:
                    o += G2[c + 2] @ vm.get(c + 2, Z)
            outs.append(o)
        return np.concatenate(outs, axis=0)

    def in_sens(c_lo, c_hi, ci):
        """dict of v sensitivities to in_{ci}, propagated over [c_lo, c_hi]."""
        return _propagate_v(T, F1, F2, c_lo, c_hi, SIZE, Z48, Z48,
                            {ci: I48})

    anchor_tiles = sorted(set(a for a in anc if a is not None))
    chain = []
    prev_a = None
    for a in anchor_tiles:
        sc = spans[a][0]
        terms = []
        if prev_a is None:
            lo = 2
        else:
            lo = spans[prev_a][0]
            vm1 = np.concatenate([I48, Z48], axis=1)
            vm2 = np.concatenate([Z48, I48], axis=1)
            vm = _propagate_v(T, F1, F2, lo, sc - 1, 96, vm1, vm2, {})
            mat = np.concatenate([vm[sc - 1], vm[sc - 2]], axis=0)
            terms.append(("state", prev_a, mat.T))
        for ci in range(lo, sc):
            vm = in_sens(max(lo, 2), sc - 1, ci)
            mat = np.concatenate([vm[sc - 1], vm[sc - 2]], axis=0)
            if np.any(mat):
                terms.append(("in", ci, mat.T))
        chain.append({"anchor": a, "terms": terms})
        prev_a = a

    band = []
    for t in range(NT):
        cs, ce = spans[t]
        r0 = P * t
        a = anc[t]
        terms = []
        win_last = min(ce + 2, SIZE - 1)
        prop_last = min(win_last, SIZE - 3)
        if a is None:
            win_first = 0 if t == 0 else 2
        else:
            win_first = spans[a][0]
            vm1 = np.concatenate([I48, Z48], axis=1)
            vm2 = np.concatenate([Z48, I48], axis=1)
            vm = _propagate_v(T, F1, F2, win_first, prop_last, 96, vm1, vm2, {})
            outm = out_rows(vm, {}, range(cs, ce + 1), 96)
            terms.append(("state", a, outm[r0 - 48 * cs: r0 - 48 * cs + P].T))
        for ci in range(win_first, win_last + 1):
            vm = in_sens(max(win_first, 2), prop_last, ci) if 2 <= ci <= SIZE - 3 else {}
            outm = out_rows(vm, {ci: I48}, range(cs, ce + 1), SIZE)
            sub = outm[r0 - 48 * cs: r0 - 48 * cs + P]
            if np.any(sub):
                terms.append(("in", ci, sub.T))
        band.append({"tile": t, "terms": terms})

    return chain, band, anc


def _merge_terms(terms):
    """Merge consecutive in-column terms and re-split at 128-row input-tile
    boundaries, zero-padding each piece back to its tile start so every
    moving-operand slice begins at partition 0 (matmul cost is moving-width
    only, so padded contract rows are free)."""
    out_terms = [t for t in terms if t[0] == "state"]
    ins = sorted((t for t in terms if t[0] == "in"), key=lambda t: t[1])
    runs = []
    for kind, c, mat in ins:
        if runs and runs[-1][0] + len(runs[-1][1]) == c:
            runs[-1][1].append(mat)
        else:
            runs.append((c, [mat]))
    for c0, mats in runs:
        big = np.concatenate(mats, axis=0)
        r0, r1 = 48 * c0, 48 * c0 + big.shape[0]
        for kt in range(r0 // P, -(-r1 // P)):
            hi = min(r1, P * kt + P)
            piece = np.zeros((hi - P * kt, big.shape[1]))
            lo = max(r0, P * kt)
            piece[lo - P * kt:] = big[lo - r0: hi - r0]
            out_terms.append(("in", kt, piece))
    return out_terms


# ---------------------------------------------------------------------------
# Device kernel
# ---------------------------------------------------------------------------

NPAIR = 8  # adjacent-k band piece pairs demoted to fp8e4m3 DoubleRow


def _select_fp8_pairs(band_m):
    """Pick the NPAIR lowest-energy adjacent-k in-piece pairs.  fp8 on a
    pair adds ~2.6% relative error on that pair's share of the output
    energy; 8 pairs measured 9.9e-3 end-to-end vs the 2e-2 gate."""
    cands = []
    for bi, b in enumerate(band_m):
        ins = [(ti, t) for ti, t in enumerate(b["terms"]) if t[0] == "in"]
        for i in range(len(ins) - 1):
            (t1, (_, k1, m1)), (t2, (_, k2, m2)) = ins[i], ins[i + 1]
            if k2 == k1 + 1:
                e = float((m1 * m1).sum() + (m2 * m2).sum())
                cands.append((e, bi, t1, t2, k1))
    cands.sort()
    used = set()
    chosen = []
    for e, bi, t1, t2, k1 in cands:
        if len(chosen) >= NPAIR:
            break
        if (bi, t1) in used or (bi, t2) in used:
            continue
        used.update(((bi, t1), (bi, t2)))
        chosen.append((bi, t1, t2, k1))
    return chosen


def _schedule(chain, band):
    """Issue order: chain steps run 2 bands ahead of the tiles that consume
    their anchor states, so PSUM->SBUF state copies hide under band work."""
    units = [("c", 0), ("b", 0), ("c", 1), ("b", 1)]
    nb = 2
    for i in range(2, len(chain)):
        units.append(("c", i))
        units.append(("b", nb)); units.append(("b", nb + 1))
        nb += 2
    while nb < len(band):
        units.append(("b", nb)); nb += 1
    return units


def _build_device_kernel(chain_m, band_m, units, unit_cols, unit_off,
                         term_meta, np8t, nxp):
    import concourse.mybir as mybir
    from concourse import bacc
    from concourse.tile import TileContext

    f32 = mybir.dt.float32
    bf = mybir.dt.bfloat16
    fp8 = mybir.dt.float8e4

    total_cols = sum(unit_cols)
    nc = bacc.Bacc()
    xT = nc.dram_tensor("xT", [N, BS], bf, kind="ExternalInput")
    wt = nc.dram_tensor("wt", [P, total_cols], bf, kind="ExternalInput")
    wt8 = nc.dram_tensor("wt8", [np8t * P, 2 * P], fp8, kind="ExternalInput")
    x8 = nc.dram_tensor("x8", [nxp * P, 2 * BS], fp8, kind="ExternalInput")
    outT = nc.dram_tensor("outT", [N, BS], bf, kind="ExternalOutput")

    xT_r = xT.rearrange("(k p) m -> k p m", p=P)
    wt8_r = wt8.rearrange("(n p) (a b) -> n p a b", p=P, a=2)
    x8_r = x8.rearrange("(n p) (a f) -> n p a f", p=P, a=2)

    with TileContext(nc) as tc:
        with (
            tc.tile_pool(name="xpool", bufs=1) as xpool,
            tc.tile_pool(name="spool", bufs=1) as spool,
            tc.tile_pool(name="wpool", bufs=8) as wpool,
            tc.tile_pool(name="opool", bufs=3) as opool,
            tc.tile_pool(name="cps", bufs=2, space="PSUM") as cpspool,
            tc.tile_pool(name="bps", bufs=2, space="PSUM") as bpspool,
        ):
            xtiles = []
            issued = 0

            def issue_x(upto):
                nonlocal issued
                while issued < min(upto, NK):
                    xk = xpool.tile([P, BS], bf, tag=f"x{issued}",
                                    name=f"x{issued}")
                    # halves so the first matmuls (m=0) wait on 128 KB,
                    # not 256 KB, trimming the cold-start DMA latency.
                    for m in range(NM):
                        nc.scalar.dma_start(
                            out=xk[:, m * MW:(m + 1) * MW],
                            in_=xT_r[issued][:, m * MW:(m + 1) * MW])
                    xtiles.append(xk)
                    issued += 1

            # fp8 paired x tiles [128, 2, 1024]: issued upfront on the
            # SWDGE ring (idle until stores begin); consumers run much
            # later in the stream.
            x8tiles = []
            for i in range(nxp):
                x8t = xpool.tile([P, 2, BS], fp8, tag=f"p8{i}",
                                 name=f"x8_{i}")
                nc.gpsimd.dma_start(out=x8t[:], in_=x8_r[i])
                x8tiles.append(x8t)

            states = {}
            for ui, (kind, idx) in enumerate(units):
                terms = chain_m[idx]["terms"] if kind == "c" else band_m[idx]["terms"]
                metas = term_meta[ui]
                cols = unit_cols[ui]
                off0 = unit_off[ui]
                wtile = None
                if cols > 0:
                    wtile = wpool.tile([P, cols], bf, tag="w", name=f"w{ui}")
                    nc.sync.dma_start(out=wtile[:],
                                      in_=wt[:, off0:off0 + cols])
                need_x = max((m[1] + 1 for m in metas if m[0] == "in"),
                             default=0)
                issue_x(need_x)
                outw = 96 if kind == "c" else P
                pool = cpspool if kind == "c" else bpspool
                tagp = "c" if kind == "c" else "b"
                ps = {
                    m: pool.tile([outw, MW], f32, tag=f"{tagp}{m}",
                                 name=f"ps_{kind}{idx}_{m}")
                    for m in range(NM)
                }
                live = [ti for ti in range(len(terms))
                        if metas[ti][0] != "skip"]
                first_t, last_t = live[0], live[-1]
                for ti, (tkind, src, mat) in enumerate(terms):
                    mkind = metas[ti][0]
                    if mkind == "skip":
                        continue
                    if mkind == "f8":
                        blk, xpi = metas[ti][1], metas[ti][2]
                        w8t = wpool.tile([P, 2, P], fp8, tag="w8",
                                         name=f"w8_{blk}")
                        nc.sync.dma_start(out=w8t[:], in_=wt8_r[blk])
                        for m in range(NM):
                            nc.tensor.matmul(
                                ps[m][:],
                                lhsT=w8t[:],
                                rhs=x8tiles[xpi][:, :, m * MW:(m + 1) * MW],
                                start=(ti == first_t), stop=(ti == last_t),
                                perf_mode=mybir.MatmulPerfMode.DoubleRow,
                                skip_group_check=True,
                            )
                        continue
                    kk, coff = metas[ti][1], metas[ti][2]
                    kdim = mat.shape[0]
                    lhsT = wtile[:kdim, coff:coff + mat.shape[1]]
                    for m in range(NM):
                        if tkind == "state":
                            rhs = states[src][:kdim, m * MW:(m + 1) * MW]
                        else:
                            rhs = xtiles[kk][:kdim, m * MW:(m + 1) * MW]
                        nc.tensor.matmul(
                            ps[m][:], lhsT=lhsT, rhs=rhs,
                            start=(ti == first_t), stop=(ti == last_t),
                        )
                if kind == "c":
                    a = chain_m[idx]["anchor"]
                    st = spool.tile([96, BS], bf, tag=f"s{a}", name=f"s{a}")
                    states[a] = st
                    for m in range(NM):
                        nc.vector.tensor_copy(st[:, m * MW:(m + 1) * MW],
                                              ps[m][:])
                else:
                    t = band_m[idx]["tile"]
                    ot = opool.tile([P, BS], bf, tag="o", name=f"o{t}")
                    # the last few stores go out on the idle SP HWDGE ring
                    # so the kernel does not end on a long SWDGE drain.
                    last = ui == len(units) - 1
                    eng = nc.sync if ui >= len(units) - 3 else nc.gpsimd
                    for m in range(NM):
                        # drain the two PSUM chunks on different engines so
                        # they retire in parallel (DVE + ACT).
                        if m == 0:
                            nc.vector.tensor_copy(ot[:, m * MW:(m + 1) * MW],
                                                  ps[m][:])
                        else:
                            nc.scalar.copy(ot[:, m * MW:(m + 1) * MW],
                                           ps[m][:])
                        # final tile: two stores on two HWDGE rings so their
                        # ~0.6 us issue slots run in parallel at stream end.
                        e = nc.scalar if (last and m == 1) else eng
                        e.dma_start(
                            out=outT[t * P:(t + 1) * P, m * MW:(m + 1) * MW],
                            in_=ot[:, m * MW:(m + 1) * MW],
                        )
    if not nc.is_finalized():
        nc.finalize()
    return nc


_XMAJOR_IDX = None


def _xmajor_idx():
    global _XMAJOR_IDX
    if _XMAJOR_IDX is None:
        n = np.arange(N)
        _XMAJOR_IDX = (n % SIZE) * SIZE + n // SIZE
    return _XMAJOR_IDX


def kernel(inputs: np.ndarray, weights: np.ndarray) -> np.ndarray:
    from concourse.bass_utils import run_bass_kernel_spmd

    inputs = np.ascontiguousarray(inputs, dtype=np.float32)
    weights = np.ascontiguousarray(weights, dtype=np.float32)

    chain, band, _ = _build_plan(weights)
    chain_m = [{"anchor": s["anchor"], "terms": _merge_terms(s["terms"])}
               for s in chain]
    band_m = [{"tile": b["tile"], "terms": _merge_terms(b["terms"])}
              for b in band]
    units = _schedule(chain_m, band_m)

    # fp8 DoubleRow demotion of low-energy adjacent-k band piece pairs.
    chosen = _select_fp8_pairs(band_m)
    f8map = {}          # (band_idx, ti) -> role
    for bi, t1, t2, k1 in chosen:
        f8map[(bi, t1)] = ("f8", k1)
        f8map[(bi, t2)] = ("skip",)
    xp_keys = sorted(set(k1 for _, _, _, k1 in chosen))
    xp_index = {k: i for i, k in enumerate(xp_keys)}

    # Pack stationaries in consumption order: unit -> terms side by side.
    unit_cols, unit_off, term_meta, packed = [], [], [], []
    w8_blocks = []      # fp8 stationaries, unit order
    off = 0
    for kind, idx in units:
        terms = chain_m[idx]["terms"] if kind == "c" else band_m[idx]["terms"]
        metas = []
        c0 = 0
        for ti, (tkind, src, mat) in enumerate(terms):
            role = f8map.get((idx, ti)) if kind == "b" else None
            if role is not None and role[0] == "skip":
                metas.append(("skip", 0, 0))
                continue
            if role is not None:
                k1 = role[1]
                m1 = terms[ti][2]
                ti2 = ti + 1
                while terms[ti2][0] != "in" or terms[ti2][1] != k1 + 1:
                    ti2 += 1
                m2 = terms[ti2][2]
                blk = np.zeros((P, 2, P), dtype=np.float64)
                blk[:m1.shape[0], 0, :] = m1
                blk[:m2.shape[0], 1, :] = m2
                metas.append(("f8", len(w8_blocks), xp_index[k1]))
                w8_blocks.append(blk.reshape(P, 2 * P))
                continue
            metas.append((tkind, src, c0))
            c0 += mat.shape[1]
            buf = np.zeros((P, mat.shape[1]), dtype=np.float64)
            buf[:mat.shape[0]] = mat
            packed.append(buf)
        term_meta.append(metas)
        unit_off.append(off)
        unit_cols.append(c0)
        off += c0
    wt_packed = np.ascontiguousarray(
        np.concatenate(packed, axis=1).astype(bf16))
    f8 = ml_dtypes.float8_e4m3fn
    wt8_packed = np.ascontiguousarray(
        np.concatenate(w8_blocks, axis=0).astype(f8))   # (NP8T*128, 256)

    idx = _xmajor_idx()
    xP = inputs.reshape(B, SIZE, SIZE).transpose(0, 2, 1).reshape(B, N)
    xPf = xP.astype(np.float32)
    xP = xP.astype(bf16)

    nc = _build_device_kernel(chain_m, band_m, units, unit_cols, unit_off,
                              term_meta, len(w8_blocks), len(xp_keys))

    def x8_pack(core):
        xc = xPf[core * BS:(core + 1) * BS].T          # (N, BS)
        blocks = []
        for k in xp_keys:
            pair = np.empty((P, 2 * BS), dtype=np.float32)
            pair[:, :BS] = xc[P * k:P * k + P]
            pair[:, BS:] = xc[P * (k + 1):P * (k + 1) + P]
            blocks.append(pair)
        return np.ascontiguousarray(np.concatenate(blocks, axis=0).astype(f8))

    in_maps = [
        {
            "xT": np.ascontiguousarray(xP[c * BS:(c + 1) * BS].T),
            "wt": wt_packed,
            "wt8": wt8_packed,
            "x8": x8_pack(c),
        }
        for c in range(NCORES)
    ]
    trace = bool(int(os.environ.get("KERNEL_TRACE", "0")))
    res = run_bass_kernel_spmd(
        nc, in_maps, core_ids=list(range(NCORES)), trace=trace
    )
    if trace and res.exec_time_ns is not None:
        print(f"HW exec time: {res.exec_time_ns} ns")
        if res.instructions_and_trace is not None:
            print(f"trace: {res.instructions_and_trace[1]}")

    outP = np.concatenate(
        [res.results[c]["outT"].T.astype(np.float32) for c in range(NCORES)],
        axis=0,
    )
    return np.ascontiguousarray(
        outP.reshape(B, SIZE, SIZE).transpose(0, 2, 1).reshape(B, N)
    )


# revision 27
# speedup vs baseline: 1.0800x; 1.0800x over previous
"""Trainium2 kernel for nn_Graph_41609643163904.

The reference op is a sequential per-cell scatter sweep over a 48x48 grid.
Every step is linear, so the sweep is a fixed linear operator M (2304x2304)
of the weights.  M in x-major order is block-lower-triangular with NO decay
(dense 58%), so the v2/v3 dense blocked matmul costs 376 PE matmuls.

v4 exploits the exact SEMISEPARABLE structure of the cascade: all influence
of grid columns < c on columns >= c flows through a 96-dim state
s_c = [v_{c-1}; v_{c-2}], where v_c are the per-column "center read"
values.  Device work per core (1024 samples):
  * an 8-step chain computes anchor states (one per 2 output j-tiles);
    each step is ~3 matmuls (state transition + input injections),
  * each 128-row output j-tile = U_t @ s_anchor + a short banded window of
    direct input terms (~3 matmuls).
Total 200 matmuls @512 moving vs 752 for dense: ~2x less PE time.
All operands bf16 (measured end-to-end quantization error 4.3e-3 vs the
2e-2 budget); PSUM accumulates f32; output shipped bf16, upcast on host.

Data parallel over batch: 8192 samples / 8 cores, no cross-core comm.
"""

import os

import numpy as np
import ml_dtypes

SIZE = 48
D = 2
K = 5
N = SIZE * SIZE          # 2304
B = 8192
NCORES = 8
BS = B // NCORES         # 1024 samples per core

P = 128
NK = N // P              # 18 input k-tiles
NT = N // P              # 18 output j-tiles
MW = 512                 # moving width (fp32 PSUM bank limit)
NM = BS // MW            # 2 m-chunks per core

bf16 = ml_dtypes.bfloat16


# ---------------------------------------------------------------------------
# Host math: per-column operators and the state-space plan (fp64, exact).
# ---------------------------------------------------------------------------

def _col_ops(w):
    w = w.astype(np.float64)
    T, E, F1, F2, G1, G2 = {}, {}, {}, {}, {}, {}
    for c in range(2, SIZE - 2):
        L = np.zeros((SIZE, SIZE))
        for yp in range(2, SIZE - 2):
            for d in (1, 2):
                y = yp + d
                if 2 <= y <= SIZE - 3:
                    L[y, yp] = w[yp, c, 2 + d, 2]
        T[c] = np.linalg.inv(np.eye(SIZE) - L)

        Ec = np.zeros((SIZE, SIZE))
        for y in range(2, SIZE - 2):
            Ec[y, y] = w[y, c, 2, 2]
            for d in (1, 2):
                yp = y + d
                if yp <= SIZE - 3:
                    Ec[y, yp] = w[yp, c, 2 - d, 2]
        for y in (0, 1, SIZE - 2, SIZE - 1):
            Ec[y, y] = 1.0
            for yp in range(2, SIZE - 2):
                dy = y - yp
                if abs(dy) <= 2 and dy != 0:
                    Ec[y, yp] += w[yp, c, 2 + dy, 2]
        E[c] = Ec

        for mats, dx in ((F1, 3), (F2, 4), (G1, 1), (G2, 0)):
            A = np.zeros((SIZE, SIZE))
            for yp in range(2, SIZE - 2):
                for dy in range(-2, 3):
                    y = yp + dy
                    if 0 <= y < SIZE:
                        A[y, yp] = w[yp, c, 2 + dy, dx]
            mats[c] = A
    return T, E, F1, F2, G1, G2


def _propagate_v(T, F1, F2, c_from, c_to, n_rhs, vm1, vm2, inj):
    """Sensitivities of v_c (c in [c_from, c_to]) to a basis: v_{c_from-1}
    := vm1, v_{c_from-2} := vm2 (48 x n_rhs each, may be zero), plus
    per-column injections inj[c] (48 x n_rhs)."""
    Z = np.zeros((SIZE, n_rhs))
    v = {}

    def getv(c):
        if c in v:
            return v[c]
        if c == c_from - 1:
            return vm1
        if c == c_from - 2:
            return vm2
        return Z

    for c in range(c_from, c_to + 1):
        u = inj.get(c, Z)
        if c - 1 >= 2:
            u = u + F1[c - 1] @ getv(c - 1)
        if c - 2 >= 2:
            u = u + F2[c - 2] @ getv(c - 2)
        v[c] = T[c] @ u
    return v


def _build_plan(w):
    """Chain steps + band terms, all stationaries fp64 in lhsT layout
    (contract, out)."""
    T, E, F1, F2, G1, G2 = _col_ops(w)
    spans = [((P * t) // SIZE, (P * t + P - 1) // SIZE) for t in range(NT)]
    anc = [None, None] + [2 * ((t - 2) // 2) + 2 for t in range(2, NT)]

    I48 = np.eye(SIZE)
    Z48 = np.zeros((SIZE, SIZE))

    def out_rows(vm, inm, cols, n_rhs):
        Z = np.zeros((SIZE, n_rhs))
        outs = []
        for c in cols:
            if c < 2:
                o = inm.get(c, Z).copy()
                if c == 0:
                    o += G2[2] @ vm.get(2, Z)
                else:
                    o += G1[2] @ vm.get(2, Z) + G2[3] @ vm.get(3, Z)
            elif c > SIZE - 3:
                o = inm.get(c, Z).copy()
                if c == SIZE - 2:
                    o += F1[SIZE - 3] @ vm.get(SIZE - 3, Z)
                    o += F2[SIZE - 4] @ vm.get(SIZE - 4, Z)
                else:
                    o += F2[SIZE - 3] @ vm.get(SIZE - 3, Z)
            else:
                o = E[c] @ vm.get(c, Z)
                if c + 1 <= SIZE - 3:
                    o += G1[c + 1] @ vm.get(c + 1, Z)
                if c + 2 <= SIZE - 3:
                    o += G2[c + 2] @ vm.get(c + 2, Z)
            outs.append(o)
        return np.concatenate(outs, axis=0)

    def in_sens(c_lo, c_hi, ci):
        """dict of v sensitivities to in_{ci}, propagated over [c_lo, c_hi]."""
        return _propagate_v(T, F1, F2, c_lo, c_hi, SIZE, Z48, Z48,
                            {ci: I48})

    anchor_tiles = sorted(set(a for a in anc if a is not None))
    chain = []
    prev_a = None
    for a in anchor_tiles:
        sc = spans[a][0]
        terms = []
        if prev_a is None:
            lo = 2
        else:
            lo = spans[prev_a][0]
            vm1 = np.concatenate([I48, Z48], axis=1)
            vm2 = np.concatenate([Z48, I48], axis=1)
            vm = _propagate_v(T, F1, F2, lo, sc - 1, 96, vm1, vm2, {})
            mat = np.concatenate([vm[sc - 1], vm[sc - 2]], axis=0)
            terms.append(("state", prev_a, mat.T))
        for ci in range(lo, sc):
            vm = in_sens(max(lo, 2), sc - 1, ci)
            mat = np.concatenate([vm[sc - 1], vm[sc - 2]], axis=0)
            if np.any(mat):
                terms.append(("in", ci, mat.T))
        chain.append({"anchor": a, "terms": terms})
        prev_a = a

    band = []
    for t in range(NT):
        cs, ce = spans[t]
        r0 = P * t
        a = anc[t]
        terms = []
        win_last = min(ce + 2, SIZE - 1)
        prop_last = min(win_last, SIZE - 3)
        if a is None:
            win_first = 0 if t == 0 else 2
        else:
            win_first = spans[a][0]
            vm1 = np.concatenate([I48, Z48], axis=1)
            vm2 = np.concatenate([Z48, I48], axis=1)
            vm = _propagate_v(T, F1, F2, win_first, prop_last, 96, vm1, vm2, {})
            outm = out_rows(vm, {}, range(cs, ce + 1), 96)
            terms.append(("state", a, outm[r0 - 48 * cs: r0 - 48 * cs + P].T))
        for ci in range(win_first, win_last + 1):
            vm = in_sens(max(win_first, 2), prop_last, ci) if 2 <= ci <= SIZE - 3 else {}
            outm = out_rows(vm, {ci: I48}, range(cs, ce + 1), SIZE)
            sub = outm[r0 - 48 * cs: r0 - 48 * cs + P]
            if np.any(sub):
                terms.append(("in", ci, sub.T))
        band.append({"tile": t, "terms": terms})

    return chain, band, anc


def _merge_terms(terms):
    """Merge consecutive in-column terms and re-split at 128-row input-tile
    boundaries, zero-padding each piece back to its tile start so every
    moving-operand slice begins at partition 0 (matmul cost is moving-width
    only, so padded contract rows are free)."""
    out_terms = [t for t in terms if t[0] == "state"]
    ins = sorted((t for t in terms if t[0] == "in"), key=lambda t: t[1])
    runs = []
    for kind, c, mat in ins:
        if runs and runs[-1][0] + len(runs[-1][1]) == c:
            runs[-1][1].append(mat)
        else:
            runs.append((c, [mat]))
    for c0, mats in runs:
        big = np.concatenate(mats, axis=0)
        r0, r1 = 48 * c0, 48 * c0 + big.shape[0]
        for kt in range(r0 // P, -(-r1 // P)):
            hi = min(r1, P * kt + P)
            piece = np.zeros((hi - P * kt, big.shape[1]))
            lo = max(r0, P * kt)
            piece[lo - P * kt:] = big[lo - r0: hi - r0]
            out_terms.append(("in", kt, piece))
    return out_terms


# ---------------------------------------------------------------------------
# Device kernel
# ---------------------------------------------------------------------------

def _schedule(chain, band):
    """Issue order: chain steps run 2 bands ahead of the tiles that consume
    their anchor states, so PSUM->SBUF state copies hide under band work."""
    units = [("c", 0), ("b", 0), ("c", 1), ("b", 1)]
    nb = 2
    for i in range(2, len(chain)):
        units.append(("c", i))
        units.append(("b", nb)); units.append(("b", nb + 1))
        nb += 2
    while nb < len(band):
        units.append(("b", nb)); nb += 1
    return units


def _build_device_kernel(chain_m, band_m, units, unit_cols, unit_off,
                         term_meta):
    import concourse.mybir as mybir
    from concourse import bacc
    from concourse.tile import TileContext

    f32 = mybir.dt.float32
    bf = mybir.dt.bfloat16

    total_cols = sum(unit_cols)
    nc = bacc.Bacc()
    xT = nc.dram_tensor("xT", [N, BS], bf, kind="ExternalInput")
    wt = nc.dram_tensor("wt", [P, total_cols], bf, kind="ExternalInput")
    outT = nc.dram_tensor("outT", [N, BS], bf, kind="ExternalOutput")

    xT_r = xT.rearrange("(k p) m -> k p m", p=P)

    with TileContext(nc) as tc:
        with (
            tc.tile_pool(name="xpool", bufs=1) as xpool,
            tc.tile_pool(name="spool", bufs=1) as spool,
            tc.tile_pool(name="wpool", bufs=8) as wpool,
            tc.tile_pool(name="opool", bufs=3) as opool,
            tc.tile_pool(name="cps", bufs=2, space="PSUM") as cpspool,
            tc.tile_pool(name="bps", bufs=2, space="PSUM") as bpspool,
        ):
            xtiles = []
            issued = 0

            def issue_x(upto):
                nonlocal issued
                while issued < min(upto, NK):
                    xk = xpool.tile([P, BS], bf, tag=f"x{issued}",
                                    name=f"x{issued}")
                    # halves so the first matmuls (m=0) wait on 128 KB,
                    # not 256 KB, trimming the cold-start DMA latency.
                    for m in range(NM):
                        nc.scalar.dma_start(
                            out=xk[:, m * MW:(m + 1) * MW],
                            in_=xT_r[issued][:, m * MW:(m + 1) * MW])
                    xtiles.append(xk)
                    issued += 1

            states = {}
            for ui, (kind, idx) in enumerate(units):
                terms = chain_m[idx]["terms"] if kind == "c" else band_m[idx]["terms"]
                metas = term_meta[ui]
                cols = unit_cols[ui]
                off0 = unit_off[ui]
                wtile = wpool.tile([P, cols], bf, tag="w", name=f"w{ui}")
                nc.sync.dma_start(out=wtile[:],
                                  in_=wt[:, off0:off0 + cols])
                need_x = max((m[1] + 1 for m in metas if m[0] == "in"),
                             default=0)
                issue_x(need_x)
                outw = 96 if kind == "c" else P
                pool = cpspool if kind == "c" else bpspool
                tagp = "c" if kind == "c" else "b"
                ps = {
                    m: pool.tile([outw, MW], f32, tag=f"{tagp}{m}",
                                 name=f"ps_{kind}{idx}_{m}")
                    for m in range(NM)
                }
                nterm = len(terms)
                for ti, (tkind, src, mat) in enumerate(terms):
                    kk, coff = metas[ti][1], metas[ti][2]
                    kdim = mat.shape[0]
                    lhsT = wtile[:kdim, coff:coff + mat.shape[1]]
                    for m in range(NM):
                        if tkind == "state":
                            rhs = states[src][:kdim, m * MW:(m + 1) * MW]
                        else:
                            rhs = xtiles[kk][:kdim, m * MW:(m + 1) * MW]
                        nc.tensor.matmul(
                            ps[m][:], lhsT=lhsT, rhs=rhs,
                            start=(ti == 0), stop=(ti == nterm - 1),
                        )
                if kind == "c":
                    a = chain_m[idx]["anchor"]
                    st = spool.tile([96, BS], bf, tag=f"s{a}", name=f"s{a}")
                    states[a] = st
                    for m in range(NM):
                        nc.vector.tensor_copy(st[:, m * MW:(m + 1) * MW],
                                              ps[m][:])
                else:
                    t = band_m[idx]["tile"]
                    ot = opool.tile([P, BS], bf, tag="o", name=f"o{t}")
                    # the last few stores go out on the idle SP HWDGE ring
                    # so the kernel does not end on a long SWDGE drain.
                    last = ui == len(units) - 1
                    eng = nc.sync if ui >= len(units) - 3 else nc.gpsimd
                    for m in range(NM):
                        # drain the two PSUM chunks on different engines so
                        # they retire in parallel (DVE + ACT).
                        if m == 0:
                            nc.vector.tensor_copy(ot[:, m * MW:(m + 1) * MW],
                                                  ps[m][:])
                        else:
                            nc.scalar.copy(ot[:, m * MW:(m + 1) * MW],
                                           ps[m][:])
                        # final tile: two stores on two HWDGE rings so their
                        # ~0.6 us issue slots run in parallel at stream end.
                        e = nc.scalar if (last and m == 1) else eng
                        e.dma_start(
                            out=outT[t * P:(t + 1) * P, m * MW:(m + 1) * MW],
                            in_=ot[:, m * MW:(m + 1) * MW],
                        )
    if not nc.is_finalized():
        nc.finalize()
    return nc


_XMAJOR_IDX = None


def _xmajor_idx():
    global _XMAJOR_IDX
    if _XMAJOR_IDX is None:
        n = np.arange(N)
        _XMAJOR_IDX = (n % SIZE) * SIZE + n // SIZE
    return _XMAJOR_IDX


def kernel(inputs: np.ndarray, weights: np.ndarray) -> np.ndarray:
    from concourse.bass_utils import run_bass_kernel_spmd

    inputs = np.ascontiguousarray(inputs, dtype=np.float32)
    weights = np.ascontiguousarray(weights, dtype=np.float32)

    chain, band, _ = _build_plan(weights)
    chain_m = [{"anchor": s["anchor"], "terms": _merge_terms(s["terms"])}
               for s in chain]
    band_m = [{"tile": b["tile"], "terms": _merge_terms(b["terms"])}
              for b in band]
    units = _schedule(chain_m, band_m)

    # Pack stationaries in consumption order: unit -> terms side by side.
    unit_cols, unit_off, term_meta, packed = [], [], [], []
    off = 0
    for kind, idx in units:
        terms = chain_m[idx]["terms"] if kind == "c" else band_m[idx]["terms"]
        metas = []
        c0 = 0
        for tkind, src, mat in terms:
            metas.append((tkind, src, c0))
            c0 += mat.shape[1]
            buf = np.zeros((P, mat.shape[1]), dtype=np.float64)
            buf[:mat.shape[0]] = mat
            packed.append(buf)
        term_meta.append(metas)
        unit_off.append(off)
        unit_cols.append(c0)
        off += c0
    wt_packed = np.ascontiguousarray(
        np.concatenate(packed, axis=1).astype(bf16))

    idx = _xmajor_idx()
    xP = inputs.reshape(B, SIZE, SIZE).transpose(0, 2, 1).reshape(B, N)
    xP = xP.astype(bf16)

    nc = _build_device_kernel(chain_m, band_m, units, unit_cols, unit_off,
                              term_meta)
    in_maps = [
        {
            "xT": np.ascontiguousarray(xP[c * BS:(c + 1) * BS].T),
            "wt": wt_packed,
        }
        for c in range(NCORES)
    ]
    trace = bool(int(os.environ.get("KERNEL_TRACE", "0")))
    res = run_bass_kernel_spmd(
        nc, in_maps, core_ids=list(range(NCORES)), trace=trace
    )
    if trace and res.exec_time_ns is not None:
        print(f"HW exec time: {res.exec_time_ns} ns")
        if res.instructions_and_trace is not None:
            print(f"trace: {res.instructions_and_trace[1]}")

    outP = np.concatenate(
        [res.results[c]["outT"].T.astype(np.float32) for c in range(NCORES)],
        axis=0,
    )
    return np.ascontiguousarray(
        outP.reshape(B, SIZE, SIZE).transpose(0, 2, 1).reshape(B, N)
    )
